# revision 1
# baseline (speedup 1.0000x reference)
"""Causal dilated conv1d (K=3, dilation=2, N=128 channels) on Trainium2.

out[b,t,i] = sum_{j,k} x[b, t-2k, j] * weight[i,j,k] + bias[i]

Strategy (8-core SPMD, pure data parallel over batch):
  - each core handles 4 of the 32 batch rows; weight/bias replicated
  - on-chip, per batch row: PE-transpose x into a [128(j), T+4] "strip"
    (4-col zero halo on the left so the dilated taps become plain column
    offsets), then 3 accumulated float32r matmuls with 512-wide moving
    operand compute out_T[i, t] = sum_k w_k^T @ xT[:, t-2k], ACT adds the
    per-partition bias while copying PSUM->SBUF, and PE transposes the
    result back to [t, i] layout for contiguous DMA out.
"""

import os
import threading

import numpy as np

import concourse.bass as bass  # noqa: F401  (bass types used via bacc/tile)
import concourse.mybir as mybir
import concourse.tile as tile
from concourse import bacc
from concourse.bass_utils import run_bass_kernel_spmd
from concourse.masks import make_identity

P = 128
KTAPS = 3
DIL = 2
HALO = (KTAPS - 1) * DIL  # 4
NCORES = 8
B_FULL, T_FULL = 32, 8192
B_CORE = B_FULL // NCORES  # 4

FP32 = mybir.dt.float32


def build(Bc=B_CORE, T=T_FULL, chunk=2048, tap_dtype=mybir.dt.float32r):
    """Build the per-core Bass module. Same NEFF runs SPMD on all 8 cores."""
    nc = bacc.Bacc(
        "TRN2",
        target_bir_lowering=False,
        debug=False,
        enable_asserts=False,
        num_devices=NCORES,
    )
    x_d = nc.dram_tensor("x", [Bc, T, P], tap_dtype, kind="ExternalInput")
    w_d = nc.dram_tensor("w", [P, KTAPS * P], tap_dtype, kind="ExternalInput")
    b_d = nc.dram_tensor("b", [P, 1], FP32, kind="ExternalInput")
    o_d = nc.dram_tensor("o", [Bc, T, P], FP32, kind="ExternalOutput")

    x_ap, o_ap = x_d.ap(), o_d.ap()
    n_chunks = T // chunk
    SW = 512  # tap-matmul moving width (1 PSUM bank of fp32)
    S = chunk // SW  # strips per chunk
    CPS = SW // P  # 128-subtiles per strip

    with tile.TileContext(nc) as tc:
        with (
            tc.tile_pool(name="const", bufs=1) as cp,
            tc.tile_pool(name="xn", bufs=3) as xp,
            tc.tile_pool(name="strip", bufs=2) as sp,
            tc.tile_pool(name="oT", bufs=3) as otp,
            tc.tile_pool(name="oc", bufs=3) as ocp,
            tc.tile_pool(name="pxt", bufs=3, space="PSUM") as pxtp,
            tc.tile_pool(name="pacc", bufs=3, space="PSUM") as paccp,
            tc.tile_pool(name="pto", bufs=2, space="PSUM") as ptop,
        ):
            ident = cp.tile([P, P], FP32)
            make_identity(nc, ident)
            # f32r copy of the identity for the (faster) f32r transpose-in path;
            # produced via DVE copy since memset/affine_select can't emit f32r.
            ident_r = cp.tile([P, P], tap_dtype)
            nc.vector.tensor_copy(ident_r[:], ident[:])
            w_sb = cp.tile([P, KTAPS * P], tap_dtype)
            nc.sync.dma_start(w_sb[:], w_d.ap())
            bias_sb = cp.tile([P, 1], FP32)
            nc.sync.dma_start(bias_sb[:], b_d.ap())
            zhalo = cp.tile([P, HALO], FP32)
            nc.vector.memset(zhalo[:], 0.0)

            R = chunk // P  # out rows per partition in the contiguous store

            # one-chunk-delayed transpose-out state: (oTv, b, t0) of the chunk
            # whose [t,i]-restore is interleaved into the NEXT chunk's strip
            # loop, so the PE never stalls waiting for the current chunk's
            # PSUM->SBUF bias copies (in-order engine streams).
            pending = None
            oc_pending = None

            def emit_tout_group(g):
                nonlocal oc_pending
                oTv_p, b_p, t0_p = pending
                if g == 0:
                    oc_pending = ocp.tile([P, chunk], FP32, tag="oc")
                pto = ptop.tile([P, SW], FP32, tag="pto")
                for c in range(CPS):
                    r = g * CPS + c
                    nc.tensor.transpose(
                        pto[:, c * P : (c + 1) * P], oTv_p[:, r, :], ident
                    )
                if g % 2 == 0:
                    nc.scalar.copy(oc_pending[:, g * SW : (g + 1) * SW], pto[:])
                else:
                    nc.vector.tensor_copy(
                        oc_pending[:, g * SW : (g + 1) * SW], pto[:]
                    )

            def emit_out_dma():
                _, b_p, t0_p = pending
                nc.sync.dma_start(
                    o_ap[b_p, t0_p : t0_p + chunk, :].rearrange(
                        "(p f) j -> p (f j)", p=P
                    ),
                    oc_pending[:],
                )

            for b in range(Bc):
                strip = sp.tile([P, T + HALO], tap_dtype, tag="strip")
                nc.vector.tensor_copy(strip[:, 0:HALO], zhalo[:])
                for ci in range(n_chunks):
                    t0 = ci * chunk
                    # load so partition p holds x rows {t0+c*128+p}: consecutive-t
                    # 128-blocks feed the transposes directly. Split the very
                    # first load per strip so the PE can start sooner.
                    xn = xp.tile([P, chunk], tap_dtype, tag="xn")
                    if b == 0 and ci == 0:
                        for s in range(S):
                            nc.sync.dma_start(
                                xn[:, s * SW : (s + 1) * SW].rearrange(
                                    "p (c j) -> p c j", j=P
                                ),
                                x_ap[b, t0 + s * SW : t0 + (s + 1) * SW, :].rearrange(
                                    "(c p) j -> p c j", p=P
                                ),
                            )
                    else:
                        nc.sync.dma_start(
                            xn.rearrange("p (c j) -> p c j", j=P),
                            x_ap[b, t0 : t0 + chunk, :].rearrange(
                                "(c p) j -> p c j", p=P
                            ),
                        )
                    # out_T accumulator for the whole chunk: [i, t-t0]
                    oT = otp.tile([P, chunk], FP32, tag="oT")
                    for s in range(S):
                        st = t0 + s * SW
                        # --- transpose x subtiles into the strip ---
                        pxt = pxtp.tile([P, SW], tap_dtype, tag="pxt")
                        for c in range(CPS):
                            cc = s * CPS + c
                            nc.tensor.transpose(
                                pxt[:, c * P : (c + 1) * P],
                                xn[:, cc * P : (cc + 1) * P],
                                ident_r,
                            )
                        nc.vector.tensor_copy(
                            strip[:, HALO + st : HALO + st + SW], pxt[:]
                        )
                        # --- 3 dilated taps, accumulated in PSUM ---
                        pacc = paccp.tile([P, SW], FP32, tag="pacc")
                        for k in range(KTAPS):
                            off = HALO + st - DIL * k
                            nc.tensor.matmul(
                                pacc[:],
                                w_sb[:, k * P : (k + 1) * P],
                                strip[:, off : off + SW],
                                start=(k == 0),
                                stop=(k == KTAPS - 1),
                            )
                        # --- bias during PSUM->SBUF copy (bias is per-partition here) ---
                        nc.scalar.add(oT[:, s * SW : (s + 1) * SW], pacc[:], bias_sb[:])
                        # --- delayed transpose-out of the PREVIOUS chunk ---
                        if pending is not None:
                            emit_tout_group(s)
                    if pending is not None:
                        emit_out_dma()
                    # transposed-out restore of this chunk happens during the
                    # next chunk's strip loop (col of oT = p*R + r)
                    pending = (oT.rearrange("n (p r) -> n r p", p=P), b, t0)
            # epilogue: restore + store the final chunk
            for g in range(S):
                emit_tout_group(g)
            emit_out_dma()
    nc.compile()
    return nc


_cache = {}
_lock = threading.Lock()


def _get_nc():
    with _lock:
        if "nc" not in _cache:
            tap = os.environ.get("CONV_TAP_DTYPE", "float32r")
            _cache["nc"] = build(tap_dtype=getattr(mybir.dt, tap))
        return _cache["nc"]


def prep_inputs(x, weight, bias):
    # w_all[j, k*128 + i] = weight[i, j, k]
    w_all = np.ascontiguousarray(
        np.transpose(np.asarray(weight, np.float32), (1, 2, 0)).reshape(P, KTAPS * P)
    )
    b2 = np.ascontiguousarray(np.asarray(bias, np.float32).reshape(P, 1))
    return np.ascontiguousarray(np.asarray(x, np.float32)), w_all, b2


def kernel(x, weight, bias, _trace=False):
    x, w_all, b2 = prep_inputs(x, weight, bias)
    nc = _get_nc()
    in_maps = [
        {"x": x[c * B_CORE : (c + 1) * B_CORE], "w": w_all, "b": b2}
        for c in range(NCORES)
    ]
    res = run_bass_kernel_spmd(nc, in_maps, core_ids=list(range(NCORES)), trace=_trace)
    out = np.concatenate([r["o"] for r in res.results], axis=0)
    if _trace:
        kernel.last_results = res
    return out



# revision 4
# speedup vs baseline: 1.9162x; 1.9162x over previous
"""Causal dilated conv1d (K=3, dilation=2, N=128 channels) on Trainium2.

out[b,t,i] = sum_{j,k} x[b, t-2k, j] * weight[i,j,k] + bias[i]

Strategy (8-core SPMD, pure data parallel over batch):
  - each core handles 4 of the 32 batch rows; weight/bias replicated
  - x is pre-transposed on the HOST to [B, N, T] fp16, so the device
    does zero PE transposes: per batch row a [128(j), T+4] strip (4-col
    zero halo on the left turns the dilated taps into plain column
    offsets) is DMA'd straight in, and the conv is just 3 accumulated
    fp16 matmuls per 512-wide window: out_T[i,t] = sum_k w_k^T @ xT[:,t-2k].
  - bias is added during the PSUM->SBUF copy (fp16 out), rotated across
    the Scalar/Vector/GpSimd engines so no single copy engine limits PE.
  - output is written in [B, N, T] fp16 layout (contiguous per-partition
    runs) and un-transposed / upcast to fp32 on the host.
"""

import threading

import numpy as np

import concourse.bass as bass  # noqa: F401  (bass types used via bacc/tile)
import concourse.mybir as mybir
import concourse.tile as tile
from concourse import bacc
from concourse.bass_utils import run_bass_kernel_spmd

P = 128
KTAPS = 3
DIL = 2
HALO = (KTAPS - 1) * DIL  # 4
NCORES = 8
B_FULL, T_FULL = 32, 8192
B_CORE = B_FULL // NCORES  # 4

FP32 = mybir.dt.float32
FP16 = mybir.dt.float16


def build(Bc=B_CORE, T=T_FULL, chunk=2048):
    """Build the per-core Bass module. Same NEFF runs SPMD on all 8 cores."""
    nc = bacc.Bacc(
        "TRN2",
        target_bir_lowering=False,
        debug=False,
        enable_asserts=False,
        num_devices=NCORES,
    )
    x_d = nc.dram_tensor("x", [Bc, P, T], FP16, kind="ExternalInput")
    w_d = nc.dram_tensor("w", [P, KTAPS * P], FP16, kind="ExternalInput")
    b_d = nc.dram_tensor("b", [P, 1], FP32, kind="ExternalInput")
    o_d = nc.dram_tensor("o", [Bc, P, T], FP16, kind="ExternalOutput")

    x_ap, o_ap = x_d.ap(), o_d.ap()
    n_chunks = T // chunk
    SW = 512  # tap-matmul moving width (1 PSUM bank of fp32)
    S = chunk // SW  # strips per chunk

    with tile.TileContext(nc) as tc:
        with (
            tc.tile_pool(name="const", bufs=1) as cp,
            tc.tile_pool(name="strip", bufs=1) as sp,
            tc.tile_pool(name="oc", bufs=3) as ocp,
            tc.tile_pool(name="pacc", bufs=6, space="PSUM") as paccp,
        ):
            w_sb = cp.tile([P, KTAPS * P], FP16)
            nc.sync.dma_start(w_sb[:], w_d.ap())
            bias_sb = cp.tile([P, 1], FP32)
            nc.sync.dma_start(bias_sb[:], b_d.ap())

            # front-load ALL input DMAs: strips for every batch row are
            # resident simultaneously, so the input stream never queues
            # behind output-DMA doorbells on the sync engine.
            strips = []
            for b in range(Bc):
                strip = sp.tile([P, T + HALO], FP16, tag=f"strip{b}")
                nc.vector.memset(strip[:, 0:HALO], 0.0)
                for ci in range(n_chunks):
                    t0 = ci * chunk
                    if b == 0 and ci == 0:
                        # finer pieces so the PE can start sooner
                        for s in range(S):
                            nc.sync.dma_start(
                                strip[:, HALO + s * SW : HALO + (s + 1) * SW],
                                x_ap[b, :, s * SW : (s + 1) * SW],
                            )
                    else:
                        nc.sync.dma_start(
                            strip[:, HALO + t0 : HALO + t0 + chunk],
                            x_ap[b, :, t0 : t0 + chunk],
                        )
                strips.append(strip)

            # PSUM->SBUF copy-with-bias engines, rotated per strip
            copy_engines = [
                lambda o, i: nc.scalar.add(o, i, bias_sb),
                lambda o, i: nc.vector.tensor_scalar_add(o, i, bias_sb),
            ]
            cnt = 0
            for b in range(Bc):
                strip = strips[b]
                for ci in range(n_chunks):
                    t0 = ci * chunk
                    oc = ocp.tile([P, chunk], FP16, tag="oc")
                    for s in range(S):
                        st = t0 + s * SW
                        pacc = paccp.tile([P, SW], FP32, tag="pacc")
                        for k in range(KTAPS):
                            off = HALO + st - DIL * k
                            nc.tensor.matmul(
                                pacc[:],
                                w_sb[:, k * P : (k + 1) * P],
                                strip[:, off : off + SW],
                                start=(k == 0),
                                stop=(k == KTAPS - 1),
                            )
                        copy_engines[cnt % len(copy_engines)](
                            oc[:, s * SW : (s + 1) * SW], pacc[:]
                        )
                        cnt += 1
                    nc.sync.dma_start(o_ap[b, :, t0 : t0 + chunk], oc[:])
    nc.compile()
    return nc


_cache = {}
_lock = threading.Lock()


def _get_nc():
    with _lock:
        if "nc" not in _cache:
            _cache["nc"] = build()
        return _cache["nc"]


def prep_inputs(x, weight, bias):
    # x -> [B, N, T] fp16 (host transpose; device then needs no PE transposes)
    xt = np.swapaxes(np.asarray(x, np.float32), 1, 2).astype(np.float16)
    # w_all[j, k*128 + i] = weight[i, j, k]
    w_all = np.ascontiguousarray(
        np.transpose(np.asarray(weight, np.float32), (1, 2, 0)).reshape(P, KTAPS * P)
    ).astype(np.float16)
    b2 = np.ascontiguousarray(np.asarray(bias, np.float32).reshape(P, 1))
    return np.ascontiguousarray(xt), w_all, b2


def kernel(x, weight, bias, _trace=False):
    xt, w_all, b2 = prep_inputs(x, weight, bias)
    nc = _get_nc()
    in_maps = [
        {"x": xt[c * B_CORE : (c + 1) * B_CORE], "w": w_all, "b": b2}
        for c in range(NCORES)
    ]
    res = run_bass_kernel_spmd(nc, in_maps, core_ids=list(range(NCORES)), trace=_trace)
    ot = np.concatenate([r["o"] for r in res.results], axis=0)  # [B, N, T] fp16
    out = np.swapaxes(ot, 1, 2).astype(np.float32)
    if _trace:
        kernel.last_results = res
    return np.ascontiguousarray(out)
